# revision 1
# baseline (speedup 1.0000x reference)
"""AttentionPool (segment softmax-pool) Trainium2 kernel, 8 NeuronCores.

Math (reference):
    s = tanh(x @ W1 + b1) @ W2 + b2        # [N,1] scores
    e = exp(s - max(s))                    # global max shift
    out[b] = sum_{i in seg b} e_i x_i / (sum_{i in seg b} e_i + 1e-8)

Key identity: the global max shift cancels in the ratio (up to the
negligible 1e-8 term; |s| <= ||W2||_1 ~ 11 so exp never overflows), so we
compute e = exp(s) directly.  Every row's contribution is then local, and
with batch ids sorted, segments are contiguous runs.  Core c owns segments
[128c, 128(c+1)) and processes a fixed window of F rows starting at the
first row of segment 128c.  Rows of other cores' segments inside the
window self-mask: their relative id falls outside [0,128) so the one-hot
compare produces zero columns.

Per 128-row tile on device (all matmuls in float32r = full-rate PE):
    xT   = PE-transpose(x_tile)                        # [4][128d,128r]
    u    = W1_k.T @ xT_k   (accum over k)              # [128h, 512r]
    th   = tanh(u + b1)                                # ACT, per-partition bias
    s    = th_chunk.T @ W2                             # [128r, 1] per tile
    e    = exp(s + b2)                                 # ACT
    A    = (iota == brel) * e                          # DVE one-hot x e, f32r
    num += A.T @ x_tile ; den += A.T @ ones            # PSUM accumulate
Final: out = num * 1/(den + 1e-8), one [128,512] slab per core; host concat.
"""

import os
import sys

for _p in ("/opt/trn_rl_repo",):
    if os.path.isdir(_p) and _p not in sys.path:
        sys.path.append(_p)

import numpy as np
import ml_dtypes

N_CORES = 8
B = 1024
SEGS = B // N_CORES          # 128 segments owned per core
D = 512
H = 128
F = 33792                    # fixed per-core row window (264 tiles of 128)
TILES = F // 128


def build_nc(tiles=TILES, copy_split=2, repeats=1, bufs=None, host_t=False, gb=4):
    """Build the per-core Bass program. copy_split: how many of the 4
    per-block PSUM->SBUF transpose copies go to ACT (rest on DVE).
    repeats>1 re-emits the whole computation for delta-timing."""
    bufs = {**dict(x=40, xts=4, th=4, A=8, es=3, xtp=3, u=2, e=1), **(bufs or {})}
    import concourse.bacc as bacc
    import concourse.mybir as mybir
    import concourse.tile as tile

    F32 = mybir.dt.float32
    F32R = mybir.dt.float32r
    AF = mybir.ActivationFunctionType
    OP = mybir.AluOpType

    f_rows = tiles * 128
    nblocks = (tiles + 3) // 4

    nc = bacc.Bacc(None, target_bir_lowering=False)
    BF16 = mybir.dt.bfloat16
    x_d = nc.dram_tensor("x", (f_rows, D), F32, kind="ExternalInput")
    if host_t:
        xtb_d = nc.dram_tensor("xtb", (D, f_rows), BF16, kind="ExternalInput")
    brel_d = nc.dram_tensor("brel", (128, tiles), F32, kind="ExternalInput")
    w1_d = nc.dram_tensor("w1", (D, H), F32, kind="ExternalInput")
    w2_d = nc.dram_tensor("w2", (H, 2), F32, kind="ExternalInput")
    b1_d = nc.dram_tensor("b1", (H, 1), F32, kind="ExternalInput")
    b2_d = nc.dram_tensor("b2", (128, 1), F32, kind="ExternalInput")
    out_d = nc.dram_tensor("out", (SEGS, D), F32, kind="ExternalOutput")

    with tile.TileContext(nc) as tc:
        with (
            tc.tile_pool(name="const", bufs=1) as cpool,
            tc.tile_pool(name="xin", bufs=bufs["x"]) as xpool,
            tc.tile_pool(name="xts", bufs=bufs["xts"]) as xtspool,
            tc.tile_pool(name="th", bufs=bufs["th"]) as thpool,
            tc.tile_pool(name="abuild", bufs=bufs["A"]) as apool,
            tc.tile_pool(name="esb", bufs=bufs["es"]) as espool,
            tc.tile_pool(name="e4", bufs=bufs["es"]) as e4pool,
            tc.tile_pool(name="edram", bufs=bufs["es"], space="DRAM") as dramp,
            tc.tile_pool(name="fin", bufs=1) as fpool,
            tc.tile_pool(name="ps_xt", bufs=bufs["xtp"], space="PSUM") as xtpsum,
            tc.tile_pool(name="ps_u", bufs=bufs["u"], space="PSUM") as upsum,
            tc.tile_pool(name="ps_e", bufs=bufs["e"], space="PSUM") as epsum,
            tc.tile_pool(name="ps_num", bufs=1, space="PSUM") as numpsum,
            tc.tile_pool(name="ps_den", bufs=1, space="PSUM") as denpsum,
        ):
            # ---- constants ----
            w1r = cpool.tile([128, 4, H], F32R)   # chunk k of W1 at [:,k,:]
            nc.sync.dma_start(
                w1r[:], w1_d[:].bitcast(F32R).rearrange("(k p) h -> p k h", p=128)
            )
            if host_t:
                w1b = cpool.tile([128, 4, H], BF16)
                nc.vector.tensor_copy(w1b[:], w1r[:].bitcast(F32))
            w2r = cpool.tile([H, 2], F32R)
            nc.sync.dma_start(w2r[:], w2_d[:].bitcast(F32R))
            b1s = cpool.tile([H, 1], F32)
            nc.sync.dma_start(b1s[:], b1_d[:])
            b2s = cpool.tile([128, 1], F32)
            nc.sync.dma_start(b2s[:], b2_d[:])
            brel = cpool.tile([128, tiles], F32)
            nc.sync.dma_start(brel[:], brel_d[:])

            ii = cpool.tile([128, 128], mybir.dt.int32)
            nc.gpsimd.iota(ii[:], pattern=[[1, 128]], base=0, channel_multiplier=0)
            ip = cpool.tile([128, 1], mybir.dt.int32)
            nc.gpsimd.iota(ip[:], pattern=[[0, 1]], base=0, channel_multiplier=1)
            iif = cpool.tile([128, 128], F32)
            nc.vector.tensor_copy(iif[:], ii[:])
            ipf = cpool.tile([128, 1], F32)
            nc.vector.tensor_copy(ipf[:], ip[:])
            identr = cpool.tile([128, 128], F32R)
            nc.vector.tensor_scalar(identr[:], iif[:], ipf[:], None, op0=OP.is_equal)
            i1 = cpool.tile([128, 2], mybir.dt.int32)
            nc.gpsimd.iota(i1[:], pattern=[[0, 2]], base=1, channel_multiplier=0)
            onesr = cpool.tile([128, 2], F32R)
            nc.vector.tensor_copy(onesr[:], i1[:])

            num = numpsum.tile([SEGS, D], F32)
            den2 = denpsum.tile([2, 2 * SEGS], F32)

            GB = gb  # blocks per departition group
            for rep in range(repeats):
                xblk = [None] * 4
                xts = None
                pend_pass2 = []
                for t in range(tiles):
                    q = t % 4
                    blk_tiles = min(4, tiles - (t - q))  # tiles in this block
                    xt = xpool.tile([128, D], F32R, tag="x")
                    nc.sync.dma_start(xt[:], x_d[t * 128:(t + 1) * 128, :].bitcast(F32R))
                    xblk[q] = xt

                    if host_t:
                        if q == 0:
                            xts = xtspool.tile([128, 4, D], BF16, tag="xts")
                            for k in range(4):
                                nc.scalar.dma_start(
                                    xts[:, k, :],
                                    xtb_d[k * 128:(k + 1) * 128,
                                          t * 128:t * 128 + 512],
                                )
                    else:
                        if q == 0:
                            xts = xtspool.tile([128, 4, D], F32R, tag="xts")
                        # transpose 4 chunks into one PSUM bank
                        xq = xtpsum.tile([128, D], F32R, tag="xtp")
                        for k in range(4):
                            nc.tensor.transpose(
                                xq[:, k * 128:(k + 1) * 128],
                                xt[:, k * 128:(k + 1) * 128],
                                identr[:],
                            )
                        # one strided copy: chunk k -> xts[:, k, q*128:(q+1)*128]
                        dst = xts[:, :, q * 128:(q + 1) * 128]
                        src = xq[:].rearrange("p (k r) -> p k r", k=4)
                        if q < copy_split:
                            nc.scalar.copy(dst, src)
                        else:
                            nc.vector.tensor_copy(dst, src)

                    if q == blk_tiles - 1:
                        base = t - q  # first tile of block
                        nb = blk_tiles * 128
                        u = upsum.tile([H, 4 * 128], F32, tag="u")
                        w1use = w1b if host_t else w1r
                        for k in range(4):
                            nc.tensor.matmul(
                                u[:, 0:nb],
                                w1use[:, k, :],
                                xts[:, k, 0:nb],
                                start=(k == 0),
                                stop=(k == 3),
                            )
                        th = thpool.tile([H, 4 * 128], F32R, tag="th")
                        nc.scalar.activation(
                            th[:, 0:nb], u[:, 0:nb],
                            AF.Tanh, bias=b1s[:], scale=1.0,
                        )
                        # scores for the whole block: [2, nb] (w2 stationary)
                        ep2 = epsum.tile([2, 4 * 128], F32, tag="e")
                        nc.tensor.matmul(
                            ep2[:, 0:nb], w2r[:], th[:, 0:nb],
                            start=True, stop=True, skip_group_check=True,
                        )
                        blk = base // 4          # block index
                        g = blk % GB                # slot within group
                        if g == 0:
                            es1 = espool.tile([1, GB * 512], F32, tag="es")
                        nc.scalar.activation(
                            es1[0:1, g * 512:g * 512 + nb], ep2[0:1, 0:nb],
                            AF.Exp, bias=b2s[0:1, :], scale=1.0,
                        )
                        pend_pass2.append((base, blk_tiles, list(xblk)))
                        last_of_group = (g == GB - 1) or (t == tiles - 1)
                        if last_of_group:
                            gn = g * 512 + nb    # valid scalars in group
                            gt = (gn + 127) // 128
                            # departition e: [1, gn] -> [128, gt] via a DRAM
                            # bounce (AP balancer cannot split partition 0
                            # into 128 partitions in one hop)
                            ed = dramp.tile([GB * 512], F32, tag="ed")
                            nc.scalar.dma_start(ed[0:gn], es1[0:1, 0:gn])
                            e4 = e4pool.tile([128, GB * 4], F32, tag="e4")
                            nc.scalar.dma_start(
                                e4[:, 0:gt],
                                ed[0:gn].rearrange("(qq p) -> p qq", p=128),
                            )
                            for base2, bt2, xbl2 in pend_pass2:
                                for q2 in range(bt2):
                                    tt = base2 + q2
                                    ecol = tt - (base2 // 4 // GB) * (GB * 4)
                                    if tt % 2 == 0:
                                        A2 = apool.tile([128, 2, SEGS], F32R, tag="A")
                                    A = A2[:, tt % 2, :]
                                    nc.vector.tensor_scalar(
                                        A, iif[:], brel[:, tt:tt + 1],
                                        e4[:, ecol:ecol + 1],
                                        op0=OP.is_equal, op1=OP.mult,
                                    )
                                    nc.tensor.matmul(
                                        num[:], A, xbl2[q2][:],
                                        start=(tt == 0), stop=(tt == tiles - 1),
                                        skip_group_check=True,
                                    )
                                    if tt % 2 == 1 or tt == tiles - 1:
                                        dn = 2 if tt % 2 == 1 else 1
                                        dslice = (A2[:, 0:2, :] if dn == 2
                                                  else A2[:, 0:1, :])
                                        nc.tensor.matmul(
                                            den2[:], onesr[:], dslice,
                                            start=(tt <= 1),
                                            stop=(tt >= tiles - 2),
                                            skip_group_check=True,
                                        )
                            pend_pass2 = []

                dtmp = fpool.tile([2, 2 * SEGS], F32, tag="dtmp")
                nc.vector.tensor_copy(dtmp[0:2, :], den2[0:2, :])
                den_sb = fpool.tile([2, SEGS], F32R, tag="densb")
                nc.vector.tensor_add(den_sb[0:2, :], dtmp[0:2, 0:SEGS],
                                     dtmp[0:2, SEGS:2 * SEGS])
                pden_t = epsum.tile([SEGS, 2], F32R, tag="e")
                nc.tensor.transpose(pden_t[:, 0:2], den_sb[0:2, :], identr[0:2, 0:2])
                dsb = fpool.tile([SEGS, 1], F32)
                nc.vector.tensor_scalar(dsb[:], pden_t[:, 0:1].bitcast(F32), 1e-8, None, op0=mybir.AluOpType.add)
                rec = fpool.tile([SEGS, 1], F32)
                nc.vector.reciprocal(rec[:], dsb[:])
                osb = fpool.tile([SEGS, D], F32)
                nc.vector.tensor_scalar(osb[:], num[:], rec[:], None, op0=mybir.AluOpType.mult)
                nc.sync.dma_start(out_d[:], osb[:])

    nc.compile()
    return nc


_NC_CACHE = {}


def get_nc(tiles=TILES):
    if tiles not in _NC_CACHE:
        _NC_CACHE[tiles] = build_nc(tiles)
    return _NC_CACHE[tiles]


def make_in_maps(x, batch, W1, b1, W2, b2, tiles=TILES, n_cores=N_CORES):
    """Host-side sharding: segment-aligned fixed windows + relative ids."""
    x = np.ascontiguousarray(np.asarray(x, dtype=np.float32))
    batch = np.asarray(batch).astype(np.int64)
    W1 = np.ascontiguousarray(np.asarray(W1, dtype=np.float32))
    b1 = np.asarray(b1, dtype=np.float32).reshape(H, 1)
    W2 = np.ascontiguousarray(
        np.repeat(np.asarray(W2, dtype=np.float32).reshape(H, 1), 2, axis=1))
    b2v = float(np.asarray(b2, dtype=np.float32).reshape(-1)[0])
    b2a = np.full((128, 1), b2v, dtype=np.float32)

    n = x.shape[0]
    f_rows = tiles * 128
    bounds = np.searchsorted(batch, np.arange(0, n_cores + 1) * SEGS)
    owned = np.diff(bounds)
    if owned.max() > f_rows:
        return None  # caller falls back
    pad_to = int(bounds[:-1].max() + f_rows)
    if pad_to > n:
        xp = np.concatenate([x, np.zeros((pad_to - n, D), np.float32)], axis=0)
    else:
        xp = x
    in_maps = []
    for c in range(n_cores):
        o = int(bounds[c])
        xs = np.ascontiguousarray(xp[o:o + f_rows])
        xtb = np.ascontiguousarray(xs.T.astype(ml_dtypes.bfloat16))
        nb = min(f_rows, n - o) if n > o else 0
        br = np.full(f_rows, -1.0, dtype=np.float32)
        br[:nb] = batch[o:o + nb].astype(np.float32) - c * SEGS
        brel2d = np.ascontiguousarray(br.reshape(tiles, 128).T)
        in_maps.append({
            "x": xs, "xtb": xtb, "brel": brel2d, "w1": W1, "w2": W2,
            "b1": b1, "b2": b2a,
        })
    return in_maps


def _numpy_fallback(x, batch, W1, b1, W2, b2):
    x = np.asarray(x, dtype=np.float32)
    batch = np.asarray(batch).astype(np.int64)
    scores = np.tanh(x @ W1 + b1) @ W2 + b2
    scores = scores - scores.max()
    e = np.exp(scores)
    den = np.zeros((B, 1), np.float32)
    np.add.at(den, batch, e)
    w = e / (den[batch] + 1e-8)
    out = np.zeros((B, D), np.float32)
    np.add.at(out, batch, w * x)
    return out


_RUNNER = {}


def _make_runner(nc, n_cores):
    """Reusable jitted SPMD executable (no donation) so repeated kernel()
    calls skip NEFF/XLA recompilation."""
    import jax
    import concourse.mybir as mybir
    from jax.sharding import Mesh, PartitionSpec, NamedSharding
    from jax.experimental.shard_map import shard_map
    from concourse import bass2jax

    bass2jax.install_neuronx_cc_hook()
    partition_name = (nc.partition_id_tensor.name
                      if nc.partition_id_tensor else None)
    in_names, out_names, out_avals, zero_outs = [], [], [], []
    for alloc in nc.m.functions[0].allocations:
        if not isinstance(alloc, mybir.MemoryLocationSet):
            continue
        name = alloc.memorylocations[0].name
        if alloc.kind == "ExternalInput":
            if name != partition_name:
                in_names.append(name)
        elif alloc.kind == "ExternalOutput":
            shape = tuple(alloc.tensor_shape)
            dtype = mybir.dt.np(alloc.dtype)
            out_names.append(name)
            out_avals.append(jax.core.ShapedArray(shape, dtype))
            zero_outs.append(np.zeros(shape, dtype))
    n_params = len(in_names)
    all_in_names = list(in_names) + list(out_names)
    if partition_name is not None:
        all_in_names.append(partition_name)

    def _body(*args):
        operands = list(args)
        if partition_name is not None:
            operands.append(bass2jax.partition_id_tensor())
        outs = bass2jax._bass_exec_p.bind(
            *operands,
            out_avals=tuple(out_avals),
            in_names=tuple(all_in_names),
            out_names=tuple(out_names),
            lowering_input_output_aliases=(),
            sim_require_finite=True,
            sim_require_nnan=True,
            nc=nc,
        )
        return tuple(outs)

    devices = jax.devices()[:n_cores]
    mesh = Mesh(np.asarray(devices), ("core",))
    nspec = n_params + len(out_names)
    fn = jax.jit(
        shard_map(_body, mesh=mesh,
                  in_specs=(PartitionSpec("core"),) * nspec,
                  out_specs=(PartitionSpec("core"),) * len(out_names),
                  check_rep=False),
        keep_unused=True,
    )
    sharding = NamedSharding(mesh, PartitionSpec("core"))
    concat_zero = [
        np.zeros((n_cores * z.shape[0], *z.shape[1:]), z.dtype) for z in zero_outs
    ]
    zero_dev = [jax.device_put(a, sharding) for a in concat_zero]
    return dict(fn=fn, in_names=in_names, out_names=out_names,
                out_avals=out_avals, zero_dev=zero_dev, sharding=sharding)


def _run_fast(nc, in_maps, n_cores):
    import jax
    if "r" not in _RUNNER:
        _RUNNER["r"] = _make_runner(nc, n_cores)
    r = _RUNNER["r"]
    concat_in = [
        np.concatenate([np.asarray(in_maps[c][name]) for c in range(n_cores)],
                       axis=0)
        for name in r["in_names"]
    ]
    dev_in = [jax.device_put(a, r["sharding"]) for a in concat_in]
    outs = r["fn"](*dev_in, *r["zero_dev"])
    jax.block_until_ready(outs)
    return [
        {name: np.asarray(outs[i]).reshape(n_cores, *r["out_avals"][i].shape)[c]
         for i, name in enumerate(r["out_names"])}
        for c in range(n_cores)
    ]


def kernel(x, batch, W1, b1, W2, b2):
    x = np.asarray(x)
    batch = np.asarray(batch)
    if (x.shape != (262144, D) or batch.shape != (262144,)
            or np.asarray(W1).shape != (D, H)):
        return _numpy_fallback(x, batch, W1, b1, W2, b2)
    if np.any(batch[:-1] > batch[1:]):
        return _numpy_fallback(x, batch, W1, b1, W2, b2)
    in_maps = make_in_maps(x, batch, W1, b1, W2, b2)
    if in_maps is None:
        return _numpy_fallback(x, batch, W1, b1, W2, b2)
    nc = get_nc()
    try:
        res = _run_fast(nc, in_maps, N_CORES)
        return np.concatenate([res[c]["out"] for c in range(N_CORES)], axis=0)
    except Exception:
        from concourse.bass_utils import run_bass_kernel_spmd
        res = run_bass_kernel_spmd(nc, in_maps, list(range(N_CORES)))
        return np.concatenate(
            [res.results[c]["out"] for c in range(N_CORES)], axis=0)


if __name__ == "__main__":
    # small self-check on synthetic data (single core, reduced tiles)
    pass



# revision 9
# speedup vs baseline: 3.1161x; 3.1161x over previous
"""AttentionPool (segment softmax-pool) Trainium2 kernel, 8 NeuronCores.

Math (reference):
    s = tanh(x @ W1 + b1) @ W2 + b2        # [N,1] scores
    e = exp(s - max(s))                    # global max shift
    out[b] = sum_{i in seg b} e_i x_i / (sum_{i in seg b} e_i + 1e-8)

The global max shift cancels in the ratio (|s| <= ||W2||_1 ~ 9 so exp
never overflows), so e = exp(s) directly.  Batch ids are sorted, so core c
owns segments [128c, 128(c+1)) and processes a fixed window of F rows
starting at the first row of segment 128c.  Rows outside the core's
segments self-mask: their relative id falls outside [0,128) so the
one-hot compare produces zero columns.

Device pipeline (per 512-row block, per core):
    u   = sum_g W18[:,g].T @dr xt8[:,g]   # fp8 DoubleRow matmuls, [H, 512]
    th  = tanh(u + b1)                    # ACT -> bf16
    ep  = w2b.T @ th                      # [1, 512] PSUM
    e   = exp(ep + b2)                    # ACT -> es1 row
per 24-tile group:
    e4  = PE-transpose each [1,128] slice of es1 -> [128, 24]  (departition)
per 128-row tile:
    A   = (iota == brel) * e              # DVE one-hot bf16
    num += A.T @ x_tile                   # bf16 moving, f32 PSUM
    den += A.T @ ones                     # 1-cycle matmul, [SEGS, 1]
Final: out = num / (den + 1e-8); host concat across cores.

Inputs are host-prepared: x in bf16 [F, 512] (pass 2), and a transposed
fp8-e4m3 copy xt8 [2, 128, 2, F] = x^T[g*256+k*128+p, c] for the DoubleRow
score matmuls (fp8 error only perturbs softmax weights; it largely cancels
in the num/den ratio).
"""

import os
import sys

for _p in ("/opt/trn_rl_repo",):
    if os.path.isdir(_p) and _p not in sys.path:
        sys.path.append(_p)

import numpy as np
import ml_dtypes

N_CORES = 8
B = 1024
SEGS = B // N_CORES          # 128 segments owned per core
D = 512
H = 128
F = 33792                    # fixed per-core row window (264 tiles of 128)
TILES = F // 128
GROUP = 24                   # tiles per DMA super-chunk / departition group
NGROUPS = TILES // GROUP     # 11


def build_nc(tiles=TILES, repeats=1, bufs=None, group=GROUP):
    """Build the per-core Bass program. repeats>1 re-emits the whole
    computation for delta-timing."""
    bufs = {**dict(x=3, x8=3, th=4, A=8, es=2, e4=2, u=2, ep=2),
            **(bufs or {})}
    import concourse.bacc as bacc
    import concourse.mybir as mybir
    import concourse.tile as tile

    F32 = mybir.dt.float32
    F32R = mybir.dt.float32r
    BF16 = mybir.dt.bfloat16
    FP8 = mybir.dt.float8e3
    AF = mybir.ActivationFunctionType
    OP = mybir.AluOpType

    f_rows = tiles * 128
    ngroups = (tiles + group - 1) // group

    nc = bacc.Bacc(None, target_bir_lowering=False)
    xb_d = nc.dram_tensor("xb", (f_rows, D), BF16, kind="ExternalInput")
    xt8_d = nc.dram_tensor("xt8", (128, 4, f_rows), FP8,
                           kind="ExternalInput")
    brl_d = nc.dram_tensor("brl", (128, tiles), F32, kind="ExternalInput")
    w18_d = nc.dram_tensor("w18", (128, 4, H), BF16, kind="ExternalInput")
    w2_d = nc.dram_tensor("w2", (H, 1), F32, kind="ExternalInput")
    b1_d = nc.dram_tensor("b1", (H, 1), F32, kind="ExternalInput")
    b2_d = nc.dram_tensor("b2", (1, 1), F32, kind="ExternalInput")
    out_d = nc.dram_tensor("out", (SEGS, D), F32, kind="ExternalOutput")

    with tile.TileContext(nc) as tc:
        with (
            tc.tile_pool(name="const", bufs=1) as cpool,
            tc.tile_pool(name="xin", bufs=bufs["x"]) as xpool,
            tc.tile_pool(name="x8in", bufs=bufs["x8"]) as x8pool,
            tc.tile_pool(name="th", bufs=bufs["th"]) as thpool,
            tc.tile_pool(name="abuild", bufs=bufs["A"]) as apool,
            tc.tile_pool(name="esb", bufs=bufs["es"]) as espool,
            tc.tile_pool(name="e4sb", bufs=bufs["e4"]) as e4pool,
            tc.tile_pool(name="fin", bufs=1) as fpool,
            tc.tile_pool(name="ps_u", bufs=bufs["u"], space="PSUM") as upsum,
            tc.tile_pool(name="ps_ep", bufs=bufs["ep"], space="PSUM") as eppsum,
            tc.tile_pool(name="ps_e4", bufs=1, space="PSUM") as e4psum,
            tc.tile_pool(name="ps_num", bufs=1, space="PSUM") as numpsum,
            tc.tile_pool(name="ps_den", bufs=1, space="PSUM") as denpsum,
        ):
            # ---- constants ----
            w18 = cpool.tile([128, 4, H], BF16)
            nc.sync.dma_start(w18[:], w18_d[:])
            w2b = cpool.tile([H, 1], BF16)
            w2f = cpool.tile([H, 1], F32)
            nc.sync.dma_start(w2f[:], w2_d[:])
            nc.vector.tensor_copy(w2b[:], w2f[:])
            b1s = cpool.tile([H, 1], F32)
            nc.sync.dma_start(b1s[:], b1_d[:])
            b2s = cpool.tile([1, 1], F32)
            nc.sync.dma_start(b2s[:], b2_d[:])
            brl = cpool.tile([128, tiles], F32)
            nc.sync.dma_start(brl[:], brl_d[:])

            ii = cpool.tile([128, 128], mybir.dt.int32)
            nc.gpsimd.iota(ii[:], pattern=[[1, 128]], base=0,
                           channel_multiplier=0)
            iifb = cpool.tile([128, 128], BF16)
            nc.vector.tensor_copy(iifb[:], ii[:])
            i1 = cpool.tile([128, 1], mybir.dt.int32)
            nc.gpsimd.iota(i1[:], pattern=[[0, 1]], base=1,
                           channel_multiplier=0)
            onesb = cpool.tile([128, 1], BF16)
            nc.vector.tensor_copy(onesb[:], i1[:])
            # [1,1] identity (value 1.0) for the [1,128]->[128,1] transposes
            identb = cpool.tile([1, 1], BF16)
            nc.vector.tensor_copy(identb[:], i1[0:1, :])

            num = numpsum.tile([SEGS, D], F32)
            den = denpsum.tile([SEGS, 1], F32)

            for rep in range(repeats):
                pend = []

                def flush(entry):
                    s0, es1_t, xs_t, gt = entry
                    e4p = e4psum.tile([128, group, 2], BF16, tag="e4p")
                    for t in range(gt):
                        nc.tensor.transpose(
                            e4p[:, t, 0:1],
                            es1_t[0:1, t * 128:(t + 1) * 128],
                            identb[:],
                        )
                    e4sb = e4pool.tile([128, group], F32, tag="e4")
                    nc.vector.tensor_copy(e4sb[:, 0:gt], e4p[:, 0:gt, 0])
                    for t in range(gt):
                        T = s0 * group + t
                        A = apool.tile([128, SEGS], BF16, tag="A")
                        nc.vector.tensor_scalar(
                            A[:], iifb[:], brl[:, T:T + 1],
                            e4sb[:, t:t + 1],
                            op0=OP.is_equal, op1=OP.mult,
                        )
                        nc.tensor.matmul(
                            num[:], A[:], xs_t[:, t, :],
                            start=(T == 0), stop=(T == tiles - 1),
                            skip_group_check=True,
                        )
                        nc.tensor.matmul(
                            den[:], A[:], onesb[:],
                            start=(T == 0), stop=(T == tiles - 1),
                            skip_group_check=True,
                        )

                for s in range(ngroups):
                    gt = min(group, tiles - s * group)   # tiles this group
                    gb = (gt + 3) // 4                   # blocks this group
                    xs = xpool.tile([128, group, D], BF16, tag="x")
                    nc.sync.dma_start(
                        xs[:, 0:gt, :],
                        xb_d[s * group * 128:(s * group + gt) * 128, :]
                        .rearrange("(q p) d -> p q d", p=128),
                    )
                    x8s = x8pool.tile([128, 4, group * 128], FP8, tag="x8")
                    nc.sync.dma_start(
                        x8s[:, :, 0:gt * 128],
                        xt8_d[:, :, s * group * 128:(s * group + gt) * 128],
                    )
                    es1 = espool.tile([1, group * 128], BF16, tag="es")
                    for b in range(gb):
                        nb = min(512, gt * 128 - b * 512)
                        u = upsum.tile([H, 512], F32, tag="u")
                        for k in range(4):
                            nc.tensor.matmul(
                                u[:, 0:nb],
                                w18[:, k, :],
                                x8s[:, k, b * 512:b * 512 + nb],
                                start=(k == 0), stop=(k == 3),
                            )
                        th = thpool.tile([H, 512], BF16, tag="th")
                        nc.scalar.activation(
                            th[:, 0:nb], u[:, 0:nb],
                            AF.Tanh, bias=b1s[:], scale=1.0,
                        )
                        ep = eppsum.tile([1, 512], F32, tag="ep")
                        nc.tensor.matmul(
                            ep[:, 0:nb], w2b[:], th[:, 0:nb],
                            start=True, stop=True, skip_group_check=True,
                        )
                        nc.scalar.activation(
                            es1[0:1, b * 512:b * 512 + nb], ep[0:1, 0:nb],
                            AF.Exp, bias=b2s[0:1, :], scale=1.0,
                        )
                    pend.append((s, es1, xs, gt))
                    if len(pend) > 1:
                        flush(pend.pop(0))
                flush(pend.pop(0))

                dsb = fpool.tile([SEGS, 1], F32, tag="dsb")
                nc.vector.tensor_scalar(dsb[:], den[:], 1e-8, None,
                                        op0=OP.add)
                rec = fpool.tile([SEGS, 1], F32, tag="rec")
                nc.vector.reciprocal(rec[:], dsb[:])
                osb = fpool.tile([SEGS, D], F32, tag="osb")
                nc.vector.tensor_scalar(osb[:], num[:], rec[:], None,
                                        op0=OP.mult)
                nc.sync.dma_start(out_d[:], osb[:])

    nc.compile()
    return nc


_NC_CACHE = {}


def get_nc(tiles=TILES):
    if tiles not in _NC_CACHE:
        _NC_CACHE[tiles] = build_nc(tiles)
    return _NC_CACHE[tiles]


def make_in_maps(x, batch, W1, b1, W2, b2, tiles=TILES, n_cores=N_CORES):
    """Host-side sharding: segment-aligned fixed windows + relative ids."""
    x = np.ascontiguousarray(np.asarray(x, dtype=np.float32))
    batch = np.asarray(batch).astype(np.int64)
    W1 = np.ascontiguousarray(np.asarray(W1, dtype=np.float32))
    b1 = np.asarray(b1, dtype=np.float32).reshape(H, 1)
    W2 = np.ascontiguousarray(np.asarray(W2, dtype=np.float32).reshape(H, 1))
    b2a = np.asarray(b2, dtype=np.float32).reshape(1, 1)
    fp8 = ml_dtypes.float8_e3m4

    # W18[p, k, h] = W1[k*128 + p, h]
    W18 = np.ascontiguousarray(
        W1.reshape(4, 128, H).transpose(1, 0, 2).astype(ml_dtypes.bfloat16))

    n = x.shape[0]
    f_rows = tiles * 128
    bounds = np.searchsorted(batch, np.arange(0, n_cores + 1) * SEGS)
    owned = np.diff(bounds)
    if owned.max() > f_rows:
        return None  # caller falls back
    pad_to = int(bounds[:-1].max() + f_rows)
    if pad_to > n:
        xp = np.concatenate([x, np.zeros((pad_to - n, D), np.float32)],
                            axis=0)
    else:
        xp = x
    in_maps = []
    for c in range(n_cores):
        o = int(bounds[c])
        xs = xp[o:o + f_rows]
        xbb = np.ascontiguousarray(xs.astype(ml_dtypes.bfloat16))
        # xt8[g, p, k, c] = x^T[g*256 + k*128 + p, c]
        # xt8[p, k, c] = x^T[k*128 + p, c]
        xt8 = np.ascontiguousarray(
            xs.T.reshape(4, 128, f_rows).transpose(1, 0, 2).astype(fp8))
        nb = min(f_rows, n - o) if n > o else 0
        br = np.full(f_rows, -1.0, dtype=np.float32)
        br[:nb] = batch[o:o + nb].astype(np.float32) - c * SEGS
        brl2d = np.ascontiguousarray(
            br.reshape(tiles, 128).T)
        in_maps.append({
            "xb": xbb, "xt8": xt8, "brl": brl2d, "w18": W18, "w2": W2,
            "b1": b1, "b2": b2a,
        })
    return in_maps


def _numpy_fallback(x, batch, W1, b1, W2, b2):
    x = np.asarray(x, dtype=np.float32)
    batch = np.asarray(batch).astype(np.int64)
    scores = np.tanh(x @ W1 + b1) @ W2 + b2
    scores = scores - scores.max()
    e = np.exp(scores)
    den = np.zeros((B, 1), np.float32)
    np.add.at(den, batch, e)
    w = e / (den[batch] + 1e-8)
    out = np.zeros((B, D), np.float32)
    np.add.at(out, batch, w * x)
    return out


_RUNNER = {}


def _make_runner(nc, n_cores):
    """Reusable jitted SPMD executable (no donation) so repeated kernel()
    calls skip NEFF/XLA recompilation."""
    import jax
    import concourse.mybir as mybir
    from jax.sharding import Mesh, PartitionSpec, NamedSharding
    from jax.experimental.shard_map import shard_map
    from concourse import bass2jax

    bass2jax.install_neuronx_cc_hook()
    partition_name = (nc.partition_id_tensor.name
                      if nc.partition_id_tensor else None)
    in_names, out_names, out_avals, zero_outs = [], [], [], []
    for alloc in nc.m.functions[0].allocations:
        if not isinstance(alloc, mybir.MemoryLocationSet):
            continue
        name = alloc.memorylocations[0].name
        if alloc.kind == "ExternalInput":
            if name != partition_name:
                in_names.append(name)
        elif alloc.kind == "ExternalOutput":
            shape = tuple(alloc.tensor_shape)
            dtype = mybir.dt.np(alloc.dtype)
            out_names.append(name)
            out_avals.append(jax.core.ShapedArray(shape, dtype))
            zero_outs.append(np.zeros(shape, dtype))
    n_params = len(in_names)
    all_in_names = list(in_names) + list(out_names)
    if partition_name is not None:
        all_in_names.append(partition_name)

    def _body(*args):
        operands = list(args)
        if partition_name is not None:
            operands.append(bass2jax.partition_id_tensor())
        outs = bass2jax._bass_exec_p.bind(
            *operands,
            out_avals=tuple(out_avals),
            in_names=tuple(all_in_names),
            out_names=tuple(out_names),
            lowering_input_output_aliases=(),
            sim_require_finite=True,
            sim_require_nnan=True,
            nc=nc,
        )
        return tuple(outs)

    devices = jax.devices()[:n_cores]
    mesh = Mesh(np.asarray(devices), ("core",))
    nspec = n_params + len(out_names)
    fn = jax.jit(
        shard_map(_body, mesh=mesh,
                  in_specs=(PartitionSpec("core"),) * nspec,
                  out_specs=(PartitionSpec("core"),) * len(out_names),
                  check_rep=False),
        keep_unused=True,
    )
    sharding = NamedSharding(mesh, PartitionSpec("core"))
    concat_zero = [
        np.zeros((n_cores * z.shape[0], *z.shape[1:]), z.dtype)
        for z in zero_outs
    ]
    zero_dev = [jax.device_put(a, sharding) for a in concat_zero]
    return dict(fn=fn, in_names=in_names, out_names=out_names,
                out_avals=out_avals, zero_dev=zero_dev, sharding=sharding)


def _run_fast(nc, in_maps, n_cores):
    import jax
    if "r" not in _RUNNER:
        _RUNNER["r"] = _make_runner(nc, n_cores)
    r = _RUNNER["r"]
    concat_in = [
        np.concatenate([np.asarray(in_maps[c][name]) for c in range(n_cores)],
                       axis=0)
        for name in r["in_names"]
    ]
    dev_in = [jax.device_put(a, r["sharding"]) for a in concat_in]
    outs = r["fn"](*dev_in, *r["zero_dev"])
    jax.block_until_ready(outs)
    return [
        {name: np.asarray(outs[i]).reshape(n_cores, *r["out_avals"][i].shape)[c]
         for i, name in enumerate(r["out_names"])}
        for c in range(n_cores)
    ]


def kernel(x, batch, W1, b1, W2, b2):
    x = np.asarray(x)
    batch = np.asarray(batch)
    if (x.shape != (262144, D) or batch.shape != (262144,)
            or np.asarray(W1).shape != (D, H)):
        return _numpy_fallback(x, batch, W1, b1, W2, b2)
    if np.any(batch[:-1] > batch[1:]):
        return _numpy_fallback(x, batch, W1, b1, W2, b2)
    in_maps = make_in_maps(x, batch, W1, b1, W2, b2)
    if in_maps is None:
        return _numpy_fallback(x, batch, W1, b1, W2, b2)
    nc = get_nc()
    try:
        res = _run_fast(nc, in_maps, N_CORES)
        return np.concatenate([res[c]["out"] for c in range(N_CORES)], axis=0)
    except Exception:
        from concourse.bass_utils import run_bass_kernel_spmd
        res = run_bass_kernel_spmd(nc, in_maps, list(range(N_CORES)))
        return np.concatenate(
            [res.results[c]["out"] for c in range(N_CORES)], axis=0)


if __name__ == "__main__":
    pass
